# revision 9
# baseline (speedup 1.0000x reference)
"""Bahdanau attention (gumbel-softmax) Trainium2 kernel.

Data-parallel over the batch dim N across 8 NeuronCores (4 batches per core).
Per batch, a single pass over `key` (the only large tensor), fp16 on-chip
compute with fp32 PSUM accumulation:

  per 128-row key tile (t-tile):
    PE:  kp[u,t]   = w_k.T @ keyT                      (contract c, fp32 accum)
    ACT: tanh_ut   = tanh(kp + qp[n] + b)              (fused per-partition bias,
                                                        fp16 out)
    PE:  score[t,2]= tanh_ut.T @ [v v]                 (contract u, fp32 out)
    ACT: e[t]      = exp(score + gumbel[t] - 10)       (shift keeps e in fp16
                                                        range; cancels in softmax)
    PE:  ctx[1,c] += e16.T @ key_tile                  (online softmax numerator)
  batch epilogue: denom = sum(e); ctx/denom, e/denom; DMA out.

The kp matmul contracts over channels (needs key as [c, t]) while the context
matmul contracts over time (needs key as [t, c]), so the host ships key in
both layouts, cast to fp16 — 2 x 16.8 MiB per core, the same byte volume as
the single fp32 copy, halving effective HBM pressure per layout
(target_regime=memory). All reductions accumulate in fp32; align is produced
from the fp32 exp. The gumbel shift (-10) cancels exactly in the softmax
normalization.

Host side does only the tiny query-path projection (q @ w_conv @ w_q + b,
~6 MFLOP on 32x256 data), dtype/layout preparation, and output gather.
"""

import numpy as np
from contextlib import ExitStack

from concourse import bacc, mybir, tile
from concourse.bass_utils import run_bass_kernel_spmd

F32 = mybir.dt.float32
F16 = mybir.dt.float16
AF = mybir.ActivationFunctionType
AX = mybir.AxisListType

N, T, C, U = 32, 8192, 256, 128
NCORES = 8
NB = N // NCORES            # batches per core
P = 128                     # partition / tile size
NT = T // P                 # 64 t-tiles per batch
GROUP = 4                   # t-tiles per compute group
NG = NT // GROUP            # 16 groups per batch
BLK = 8                     # t-tiles per DMA block (0.5 MiB fp16 per stream)
NBLK = NT // BLK            # 8 DMA blocks per batch

GSHIFT = 10.0               # gumbel shift (applied on host) for fp16 exp range


def _emit_kernel(ctx: ExitStack, tc, key_d, keyt_d, wk_d, v_d, qpb_d, gum_d,
                 ctx_d, aln_d):
    nc = tc.nc

    const = ctx.enter_context(tc.tile_pool(name="const", bufs=1))
    wk_sb = const.tile([P, 2, U], F16)          # [c-in-chunk, c-chunk, u]
    nc.sync.dma_start(wk_sb[:, 0, :], wk_d[0:P, :])
    nc.sync.dma_start(wk_sb[:, 1, :], wk_d[P : 2 * P, :])
    v2_sb = const.tile([U, 2], F16)
    nc.sync.dma_start(v2_sb[:], v_d[:])
    qpb_sb = const.tile([U, NB], F32)
    nc.sync.dma_start(qpb_sb[:], qpb_d[:])
    ones_col = const.tile([P, 1], F32)
    nc.gpsimd.memset(ones_col[:], 1.0)

    gum_pool = ctx.enter_context(tc.tile_pool(name="gum", bufs=2))
    key_pool = ctx.enter_context(tc.tile_pool(name="key", bufs=3))
    keyt_pool = ctx.enter_context(tc.tile_pool(name="keyt", bufs=2))
    tanh_pool = ctx.enter_context(tc.tile_pool(name="tanh", bufs=3))
    e_pool = ctx.enter_context(tc.tile_pool(name="e", bufs=2))
    z_pool = ctx.enter_context(tc.tile_pool(name="z", bufs=2))
    aout_pool = ctx.enter_context(tc.tile_pool(name="aout", bufs=2))
    cout_pool = ctx.enter_context(tc.tile_pool(name="cout", bufs=2))
    mini_pool = ctx.enter_context(tc.tile_pool(name="mini", bufs=2))

    kp_pool = ctx.enter_context(tc.tile_pool(name="kp", bufs=3, space="PSUM"))
    sc_pool = ctx.enter_context(tc.tile_pool(name="sc", bufs=2, space="PSUM"))
    cx_pool = ctx.enter_context(tc.tile_pool(name="cx", bufs=1, space="PSUM"))
    ep_pool = ctx.enter_context(tc.tile_pool(name="ep", bufs=1, space="PSUM"))

    for n in range(NB):
        gum_sb = gum_pool.tile([P, NT], F32)
        nc.gpsimd.dma_start(gum_sb[:], gum_d[n])
        e_sb = e_pool.tile([P, NT], F32, tag="e32")
        e16_sb = e_pool.tile([P, NT], F16, tag="e16")
        cx_ps = cx_pool.tile([1, C], F32)

        key_tiles = {}
        keyt_tiles = {}
        tanh_tiles = {}

        def load_block(b):
            key_sb = key_pool.tile([P, BLK, C], F16)
            src = key_d[n, b * BLK * P : (b + 1) * BLK * P, :].rearrange(
                "(p j) c -> p j c", p=P
            )
            nc.sync.dma_start(key_sb[:], src)
            key_tiles[b] = key_sb
            if b % 2 == 0:
                keyt_sb = keyt_pool.tile([P, 2, 2 * BLK * P], F16)
                srct = keyt_d[n].rearrange("(cc p) t -> p cc t", p=P)[
                    :, :, b * BLK * P : (b + 2) * BLK * P
                ]
                nc.scalar.dma_start(keyt_sb[:], srct)
                keyt_tiles[b] = keyt_sb
                keyt_tiles[b + 1] = None  # second half of the same tile

        def front(g):
            # kp matmul + tanh for group g
            b, h = divmod(g, BLK // GROUP)
            keyt_sb = keyt_tiles[b - b % 2]
            hh = (b % 2) * (BLK // GROUP) + h
            kp_ps = kp_pool.tile([U, GROUP * P], F32)
            for cc in range(2):
                nc.tensor.matmul(
                    kp_ps[:],
                    wk_sb[:, cc, :],
                    keyt_sb[:, cc, hh * GROUP * P : (hh + 1) * GROUP * P],
                    start=(cc == 0),
                    stop=(cc == 1),
                )
            th_sb = tanh_pool.tile([U, GROUP * P], F16)
            nc.scalar.activation(
                th_sb[:], kp_ps[:], AF.Tanh, bias=qpb_sb[:, n : n + 1]
            )
            tanh_tiles[g] = th_sb

        def score(g):
            th_sb = tanh_tiles.pop(g)
            sc_ps = sc_pool.tile([P, GROUP, 2], F32)
            for j in range(GROUP):
                nc.tensor.matmul(
                    sc_ps[:, j, :],
                    th_sb[:, j * P : (j + 1) * P],
                    v2_sb[:],
                    start=(j == 0),
                    stop=(j == GROUP - 1),
                )
            z_sb = z_pool.tile([P, GROUP], F32)
            nc.vector.tensor_add(
                z_sb[:], sc_ps[:, :, 0], gum_sb[:, g * GROUP : (g + 1) * GROUP]
            )
            nc.scalar.activation(
                e_sb[:, g * GROUP : (g + 1) * GROUP], z_sb[:], AF.Exp
            )
            nc.vector.tensor_copy(
                e16_sb[:, g * GROUP : (g + 1) * GROUP],
                e_sb[:, g * GROUP : (g + 1) * GROUP],
            )

        def ctx_acc(g):
            b, h = divmod(g, BLK // GROUP)
            key_sb = key_tiles[b]
            for j in range(GROUP):
                i = g * GROUP + j
                nc.tensor.matmul(
                    cx_ps[:],
                    e16_sb[:, i : i + 1],
                    key_sb[:, h * GROUP + j, :],
                    start=(i == 0),
                    stop=(i == NT - 1),
                )

        # software-pipelined emission: PE order per iter g is
        #   kp(g) | cx(g-2) | sc(g-1)
        # so each cross-engine round-trip has a group of PE work to hide in.
        load_block(0)
        for g in range(NG + 2):
            if g < NG:
                if g % (BLK // GROUP) == 0:
                    b_next = g // (BLK // GROUP) + 1
                    if b_next < NBLK:
                        load_block(b_next)
                front(g)
            if g >= 2:
                ctx_acc(g - 2)
            if 1 <= g <= NG:
                score(g - 1)

        # batch epilogue: denominator + normalization (tiny fp32 matmuls)
        den_ps = ep_pool.tile([1, NT], F32, tag="ep")
        nc.tensor.matmul(den_ps[:], ones_col[:], e_sb[:], start=True, stop=True)
        s_sb = mini_pool.tile([1, 1], F32, tag="ssum")
        nc.vector.reduce_sum(s_sb[:], den_ps[:], axis=AX.X)
        r32_sb = mini_pool.tile([1, 1], F32, tag="recip32")
        nc.vector.reciprocal(r32_sb[:], s_sb[:])
        rb_sb = mini_pool.tile([P, 1], F32, tag="rb")
        nc.gpsimd.partition_broadcast(rb_sb[:], r32_sb[:])

        aln_sb = aout_pool.tile([P, NT], F32)
        nc.vector.tensor_scalar_mul(aln_sb[:], e_sb[:], rb_sb[:])
        nc.gpsimd.dma_start(aln_d[n], aln_sb[:])
        cxo_sb = cout_pool.tile([1, C], F32)
        nc.vector.tensor_scalar_mul(cxo_sb[:], cx_ps[:], r32_sb[:])
        nc.gpsimd.dma_start(ctx_d[n : n + 1, :], cxo_sb[:])


def build_nc():
    nc = bacc.Bacc("TRN2", target_bir_lowering=False)
    key_d = nc.dram_tensor("key", [NB, T, C], F16, kind="ExternalInput")
    keyt_d = nc.dram_tensor("keyt", [NB, C, T], F16, kind="ExternalInput")
    wk_d = nc.dram_tensor("wk", [C, U], F16, kind="ExternalInput")
    v_d = nc.dram_tensor("v", [U, 2], F16, kind="ExternalInput")
    qpb_d = nc.dram_tensor("qpb", [U, NB], F32, kind="ExternalInput")
    gum_d = nc.dram_tensor("gum", [NB, P, NT], F32, kind="ExternalInput")
    ctx_d = nc.dram_tensor("ctx", [NB, C], F32, kind="ExternalOutput")
    aln_d = nc.dram_tensor("aln", [NB, P, NT], F32, kind="ExternalOutput")
    with tile.TileContext(nc) as tc, ExitStack() as ctx:
        _emit_kernel(ctx, tc, key_d, keyt_d, wk_d, v_d, qpb_d, gum_d, ctx_d, aln_d)
    nc.compile()
    return nc


def make_in_maps(query, key, w_conv, w_q, w_k, v, b, gumbel):
    """Host-side sharding + tiny query-path projection + layout transforms."""
    query = np.asarray(query, np.float32)
    key = np.asarray(key, np.float32)
    w_conv = np.asarray(w_conv, np.float32)
    w_q = np.asarray(w_q, np.float32)
    w_k = np.asarray(w_k, np.float32)
    v = np.asarray(v, np.float32)
    b = np.asarray(b, np.float32)
    gumbel = np.asarray(gumbel, np.float32)

    qp = (query.reshape(N, C) @ w_conv.T) @ w_q + b.reshape(1, U)  # (N, U)
    # p-major rows: natural-key partition p of block b holds rows
    # t = b*1024 + p*8 + j (4KB bursts); e/gum column i = b*8 + j.
    # keyt column X = b*1024 + h*512 + jj*128 + p must hold the same row's
    # channels, i.e. key[t = b*1024 + p*8 + h*4 + jj, :].
    gum_t = np.ascontiguousarray(
        (gumbel.reshape(N, NBLK, P, BLK) - GSHIFT)
        .transpose(0, 2, 1, 3)
        .reshape(N, P, NT)
    )  # gum[n, p, b*8+j] = gumbel[n, b*1024+p*8+j] - GSHIFT
    key16 = np.ascontiguousarray(key.astype(np.float16))
    perm = (
        np.arange(T)
        .reshape(NBLK, P, 2, 4)  # [b][p][h][jj] = b*1024 + p*8 + h*4 + jj
        .transpose(0, 2, 3, 1)   # [b][h][jj][p]
        .reshape(T)
    )
    keyt16 = np.ascontiguousarray(key16[:, perm, :].transpose(0, 2, 1))
    wk16 = np.ascontiguousarray(w_k.astype(np.float16))
    v16 = np.ascontiguousarray(
        np.repeat(v.reshape(U, 1), 2, axis=1).astype(np.float16)
    )

    in_maps = []
    for i in range(NCORES):
        sl = slice(i * NB, (i + 1) * NB)
        in_maps.append(
            {
                "key": key16[sl],
                "keyt": keyt16[sl],
                "wk": wk16,
                "v": v16,
                "qpb": np.ascontiguousarray(qp[sl].T),  # (128, NB)
                "gum": np.ascontiguousarray(gum_t[sl]),
            }
        )
    return in_maps


def gather_outputs(results):
    context = np.empty((N, 1, C), np.float32)
    align = np.empty((N, 1, T), np.float32)
    for i, r in enumerate(results):
        sl = slice(i * NB, (i + 1) * NB)
        context[sl, 0, :] = r["ctx"]
        # aln[n, p, b*8+j] = align[n, b*1024 + p*8 + j]
        a = r["aln"].reshape(NB, P, NBLK, BLK).transpose(0, 2, 1, 3)
        align[sl, 0, :] = a.reshape(NB, T)
    return context, align


_NC_CACHE = None


def kernel(query, key, w_conv, w_q, w_k, v, b, gumbel):
    global _NC_CACHE
    if _NC_CACHE is None:
        _NC_CACHE = build_nc()
    in_maps = make_in_maps(query, key, w_conv, w_q, w_k, v, b, gumbel)
    res = run_bass_kernel_spmd(_NC_CACHE, in_maps, core_ids=list(range(NCORES)))
    return gather_outputs(res.results)


# revision 13
# speedup vs baseline: 1.3601x; 1.3601x over previous
"""Bahdanau attention (gumbel-softmax) Trainium2 kernel.

Data-parallel over the batch dim N across 8 NeuronCores (4 batches per core).
Per batch, a single pass over `key` (the only large tensor), fp16 on-chip
compute with fp32 PSUM accumulation:

  per 128-row key tile (t-tile):
    PE:  kp[u,t]   = w_k.T @ keyT                      (contract c, fp32 accum)
    ACT: tanh_ut   = tanh(kp + qp[n] + b)              (fused per-partition bias,
                                                        fp16 out)
    PE:  score[t,2]= tanh_ut.T @ [v v]                 (contract u, fp32 out)
    ACT: e[t]      = exp(score + gumbel[t] - 10)       (shift keeps e in fp16
                                                        range; cancels in softmax)
    PE:  ctx[1,c] += e16.T @ key_tile                  (online softmax numerator)
  batch epilogue: denom = sum(e); ctx/denom, e/denom; DMA out.

The kp matmul contracts over channels (needs key as [c, t]) while the context
matmul contracts over time (needs key as [t, c]), so the host ships key in
both layouts, cast to fp16 — 2 x 16.8 MiB per core, the same byte volume as
the single fp32 copy, halving effective HBM pressure per layout
(target_regime=memory). All reductions accumulate in fp32; align is produced
from the fp32 exp. The gumbel shift (-10) cancels exactly in the softmax
normalization.

Host side does only the tiny query-path projection (q @ w_conv @ w_q + b,
~6 MFLOP on 32x256 data), dtype/layout preparation, and output gather.
"""

import numpy as np
from contextlib import ExitStack

from concourse import bacc, mybir, tile
from concourse.bass_utils import run_bass_kernel_spmd

F32 = mybir.dt.float32
F16 = mybir.dt.float16
AF = mybir.ActivationFunctionType
AX = mybir.AxisListType

N, T, C, U = 32, 8192, 256, 128
NCORES = 8
NB = N // NCORES            # batches per core
P = 128                     # partition / tile size
NT = T // P                 # 64 t-tiles per batch
GROUP = 4                   # t-tiles per compute group
NG = NT // GROUP            # 16 groups per batch
BLK = 8                     # t-tiles per DMA block (0.5 MiB fp16 per stream)
NBLK = NT // BLK            # 8 DMA blocks per batch

GSHIFT = 10.0               # gumbel shift (applied on host) for fp16 exp range


def _emit_kernel(ctx: ExitStack, tc, key_d, keyt_d, wk_d, v_d, qpb_d, gum_d,
                 ctx_d, aln_d):
    nc = tc.nc

    const = ctx.enter_context(tc.tile_pool(name="const", bufs=1))
    wk_sb = const.tile([P, 2, U], F16)          # [c-in-chunk, c-chunk, u]
    nc.sync.dma_start(wk_sb[:, 0, :], wk_d[0:P, :])
    nc.sync.dma_start(wk_sb[:, 1, :], wk_d[P : 2 * P, :])
    v2_sb = const.tile([U, 2], F16)
    nc.sync.dma_start(v2_sb[:], v_d[:])
    qpb_sb = const.tile([U, NB], F32)
    nc.sync.dma_start(qpb_sb[:], qpb_d[:])
    ones_col = const.tile([P, 1], F32)
    nc.gpsimd.memset(ones_col[:], 1.0)

    gum_pool = ctx.enter_context(tc.tile_pool(name="gum", bufs=2))
    key_pool = ctx.enter_context(tc.tile_pool(name="key", bufs=3))
    keyt_pool = ctx.enter_context(tc.tile_pool(name="keyt", bufs=2))
    tanh_pool = ctx.enter_context(tc.tile_pool(name="tanh", bufs=3))
    e_pool = ctx.enter_context(tc.tile_pool(name="e", bufs=2))
    z_pool = ctx.enter_context(tc.tile_pool(name="z", bufs=2))
    aout_pool = ctx.enter_context(tc.tile_pool(name="aout", bufs=2))
    cout_pool = ctx.enter_context(tc.tile_pool(name="cout", bufs=2))
    mini_pool = ctx.enter_context(tc.tile_pool(name="mini", bufs=2))

    kp_pool = ctx.enter_context(tc.tile_pool(name="kp", bufs=3, space="PSUM"))
    sc_pool = ctx.enter_context(tc.tile_pool(name="sc", bufs=2, space="PSUM"))
    cx_pool = ctx.enter_context(tc.tile_pool(name="cx", bufs=2, space="PSUM"))
    ep_pool = ctx.enter_context(tc.tile_pool(name="ep", bufs=1, space="PSUM"))

    GT = NB * NG                      # 64 global groups
    GB = NB * NBLK                    # 32 global blocks
    gum_tiles, e_tiles, e16_tiles, cx_tiles = {}, {}, {}, {}
    key_tiles, keyt_tiles, tanh_tiles = {}, {}, {}

    def start_batch(n):
        gum_sb = gum_pool.tile([P, NT], F32)
        nc.sync.dma_start(gum_sb[:], gum_d[n])
        gum_tiles[n] = gum_sb
        e_tiles[n] = e_pool.tile([P, NT], F32, tag="e32", name=f"e32_{n}")
        e16_tiles[n] = e_pool.tile([P, NT], F16, tag="e16", name=f"e16_{n}")
        cx_tiles[n] = cx_pool.tile([1, C], F32, tag="cx", name=f"cx_{n}")

    def load_block(B):
        n, b = divmod(B, NBLK)
        if b == 0:
            start_batch(n)
        if b % 2 == 0:
            keyt_sb = keyt_pool.tile([P, 2, 2 * BLK * P], F16)
            srct = keyt_d[n].rearrange("(cc p) t -> p cc t", p=P)[
                :, :, b * BLK * P : (b + 2) * BLK * P
            ]
            nc.sync.dma_start(keyt_sb[:], srct)
            keyt_tiles[B] = keyt_sb
            keyt_tiles[B + 1] = None  # second half of the same tile
        key_sb = key_pool.tile([P, BLK, C], F16)
        src = key_d[n, b * BLK * P : (b + 1) * BLK * P, :].rearrange(
            "(p j) c -> p j c", p=P
        )
        nc.sync.dma_start(key_sb[:], src)
        key_tiles[B] = key_sb

    def front(G):
        # kp matmul + tanh for global group G
        n = G // NG
        B, h = divmod(G, BLK // GROUP)
        keyt_sb = keyt_tiles[B - B % 2]
        hh = (B % 2) * (BLK // GROUP) + h
        kp_ps = kp_pool.tile([U, GROUP * P], F32)
        for cc in range(2):
            nc.tensor.matmul(
                kp_ps[:],
                wk_sb[:, cc, :],
                keyt_sb[:, cc, hh * GROUP * P : (hh + 1) * GROUP * P],
                start=(cc == 0),
                stop=(cc == 1),
            )
        th_sb = tanh_pool.tile([U, GROUP * P], F16)
        nc.scalar.activation(
            th_sb[:], kp_ps[:], AF.Tanh, bias=qpb_sb[:, n : n + 1]
        )
        tanh_tiles[G] = th_sb

    def score(G):
        n, g = divmod(G, NG)
        th_sb = tanh_tiles.pop(G)
        sc_ps = sc_pool.tile([P, GROUP, 2], F32)
        for j in range(GROUP):
            nc.tensor.matmul(
                sc_ps[:, j, :],
                th_sb[:, j * P : (j + 1) * P],
                v2_sb[:],
                start=(j == 0),
                stop=(j == GROUP - 1),
            )
        z_sb = z_pool.tile([P, GROUP], F32)
        nc.vector.tensor_add(
            z_sb[:], sc_ps[:, :, 0], gum_tiles[n][:, g * GROUP : (g + 1) * GROUP]
        )
        nc.scalar.activation(
            e_tiles[n][:, g * GROUP : (g + 1) * GROUP], z_sb[:], AF.Exp
        )
        nc.vector.tensor_copy(
            e16_tiles[n][:, g * GROUP : (g + 1) * GROUP],
            e_tiles[n][:, g * GROUP : (g + 1) * GROUP],
        )

    def ctx_acc(G):
        n, g = divmod(G, NG)
        B, h = divmod(G, BLK // GROUP)
        key_sb = key_tiles[B]
        for j in range(GROUP):
            i = g * GROUP + j
            nc.tensor.matmul(
                cx_tiles[n][:],
                e16_tiles[n][:, i : i + 1],
                key_sb[:, h * GROUP + j, :],
                start=(i == 0),
                stop=(i == NT - 1),
            )

    def epilogue(n):
        # denominator + normalization (one tiny PE matmul; rest off-PE)
        e_sb = e_tiles.pop(n)
        den_ps = ep_pool.tile([1, NT], F32, tag="ep")
        nc.tensor.matmul(den_ps[:], ones_col[:], e_sb[:], start=True, stop=True)
        s_sb = mini_pool.tile([1, 1], F32, tag="ssum")
        nc.vector.reduce_sum(s_sb[:], den_ps[:], axis=AX.X)
        r32_sb = mini_pool.tile([1, 1], F32, tag="recip32")
        nc.vector.reciprocal(r32_sb[:], s_sb[:])
        rb_sb = mini_pool.tile([P, 1], F32, tag="rb")
        nc.gpsimd.partition_broadcast(rb_sb[:], r32_sb[:])
        aln_sb = aout_pool.tile([P, NT], F32)
        nc.vector.tensor_scalar_mul(aln_sb[:], e_sb[:], rb_sb[:])
        nc.sync.dma_start(aln_d[n], aln_sb[:])
        cxo_sb = cout_pool.tile([1, C], F32)
        nc.vector.tensor_scalar_mul(cxo_sb[:], cx_tiles.pop(n)[:], r32_sb[:])
        nc.sync.dma_start(ctx_d[n : n + 1, :], cxo_sb[:])

    # software-pipelined emission across ALL batches: PE order per iter G is
    #   kp(G) | cx(G-2) | sc(G-1), with block DMAs prefetched 2-3 blocks ahead.
    load_block(0)
    load_block(1)
    for G in range(GT + 2):
        if G < GT:
            if G % 2 == 0 and G // 2 + 2 < GB:
                load_block(G // 2 + 2)
            front(G)
        if G >= 2:
            ctx_acc(G - 2)
            if (G - 1) % NG == 0:
                epilogue((G - 2) // NG)
        if 1 <= G <= GT:
            score(G - 1)


def build_nc():
    nc = bacc.Bacc("TRN2", target_bir_lowering=False)
    key_d = nc.dram_tensor("key", [NB, T, C], F16, kind="ExternalInput")
    keyt_d = nc.dram_tensor("keyt", [NB, C, T], F16, kind="ExternalInput")
    wk_d = nc.dram_tensor("wk", [C, U], F16, kind="ExternalInput")
    v_d = nc.dram_tensor("v", [U, 2], F16, kind="ExternalInput")
    qpb_d = nc.dram_tensor("qpb", [U, NB], F32, kind="ExternalInput")
    gum_d = nc.dram_tensor("gum", [NB, P, NT], F32, kind="ExternalInput")
    ctx_d = nc.dram_tensor("ctx", [NB, C], F32, kind="ExternalOutput")
    aln_d = nc.dram_tensor("aln", [NB, P, NT], F32, kind="ExternalOutput")
    with tile.TileContext(nc) as tc, ExitStack() as ctx:
        _emit_kernel(ctx, tc, key_d, keyt_d, wk_d, v_d, qpb_d, gum_d, ctx_d, aln_d)
    nc.compile()
    return nc


def make_in_maps(query, key, w_conv, w_q, w_k, v, b, gumbel):
    """Host-side sharding + tiny query-path projection + layout transforms."""
    query = np.asarray(query, np.float32)
    key = np.asarray(key, np.float32)
    w_conv = np.asarray(w_conv, np.float32)
    w_q = np.asarray(w_q, np.float32)
    w_k = np.asarray(w_k, np.float32)
    v = np.asarray(v, np.float32)
    b = np.asarray(b, np.float32)
    gumbel = np.asarray(gumbel, np.float32)

    qp = (query.reshape(N, C) @ w_conv.T) @ w_q + b.reshape(1, U)  # (N, U)
    # p-major rows: natural-key partition p of block b holds rows
    # t = b*1024 + p*8 + j (4KB bursts); e/gum column i = b*8 + j.
    # keyt column X = b*1024 + h*512 + jj*128 + p must hold the same row's
    # channels, i.e. key[t = b*1024 + p*8 + h*4 + jj, :].
    gum_t = np.ascontiguousarray(
        (gumbel.reshape(N, NBLK, P, BLK) - GSHIFT)
        .transpose(0, 2, 1, 3)
        .reshape(N, P, NT)
    )  # gum[n, p, b*8+j] = gumbel[n, b*1024+p*8+j] - GSHIFT
    key16 = np.ascontiguousarray(key.astype(np.float16))
    perm = (
        np.arange(T)
        .reshape(NBLK, P, 2, 4)  # [b][p][h][jj] = b*1024 + p*8 + h*4 + jj
        .transpose(0, 2, 3, 1)   # [b][h][jj][p]
        .reshape(T)
    )
    keyt16 = np.ascontiguousarray(key16[:, perm, :].transpose(0, 2, 1))
    wk16 = np.ascontiguousarray(w_k.astype(np.float16))
    v16 = np.ascontiguousarray(
        np.repeat(v.reshape(U, 1), 2, axis=1).astype(np.float16)
    )

    in_maps = []
    for i in range(NCORES):
        sl = slice(i * NB, (i + 1) * NB)
        in_maps.append(
            {
                "key": key16[sl],
                "keyt": keyt16[sl],
                "wk": wk16,
                "v": v16,
                "qpb": np.ascontiguousarray(qp[sl].T),  # (128, NB)
                "gum": np.ascontiguousarray(gum_t[sl]),
            }
        )
    return in_maps


def gather_outputs(results):
    context = np.empty((N, 1, C), np.float32)
    align = np.empty((N, 1, T), np.float32)
    for i, r in enumerate(results):
        sl = slice(i * NB, (i + 1) * NB)
        context[sl, 0, :] = r["ctx"]
        # aln[n, p, b*8+j] = align[n, b*1024 + p*8 + j]
        a = r["aln"].reshape(NB, P, NBLK, BLK).transpose(0, 2, 1, 3)
        align[sl, 0, :] = a.reshape(NB, T)
    return context, align


_NC_CACHE = None


def kernel(query, key, w_conv, w_q, w_k, v, b, gumbel):
    global _NC_CACHE
    if _NC_CACHE is None:
        _NC_CACHE = build_nc()
    in_maps = make_in_maps(query, key, w_conv, w_q, w_k, v, b, gumbel)
    res = run_bass_kernel_spmd(_NC_CACHE, in_maps, core_ids=list(range(NCORES)))
    return gather_outputs(res.results)
